# revision 1
# baseline (speedup 1.0000x reference)
"""nn_BlockSharedRounding Trainium2 kernel.

Computes the forward of the block-shared soft rounding reference:
    a   = |x| + 0.5*tanh(delta_raw) per 32-block
    ord = searchsorted(BOUNDS, a, 'left')   (device-semantics matched)
    q   = VALUES[ord]                       (== abs_mix forward value)

Strategy: data-parallel over 8 NeuronCores (rows of x). Per core, a raw
Bass kernel streams [128, fd] fp32 chunks (fd up to 8192, small edge
chunks to shrink pipeline fill/drain) through 4 fused custom DVE ops
(abs+block-bias, low-threshold sum, ordinal, value lookup). Outputs are
written compressed — q as bf16 and ord as uint8, both exact encodings of
the 8 possible values — and the host restores the reference dtypes with
exact casts.

The comparison thresholds are b + K*ulp(b) (K=32 for b<2 else 64): the
neuron backend's eager `searchsorted` classifies values within that band
above each bound as not-greater, and the reference oracle is defined by
that backend. tanh runs on the same backend for the same reason.
"""
import numpy as np

import concourse.bass as bass
import concourse.bacc as bacc
import concourse.mybir as mybir
import concourse.dve_ops as DO
from concourse.dve_uop import DveOpSpec
from concourse.dve_spec import (
    Spec, Src0, Src1, C0, C1, C2, C3, Zero, Bin, AluOp, relu, lower,
    _has_src1, _spill_c3_to_src1,
)
from concourse.bass_utils import run_bass_kernel_spmd

# ---------------------------------------------------------------- constants
N_CORES = 8
ROWS, COLS = 4096, 8192
SHARD_ROWS = ROWS // N_CORES            # 512
SHARD_ELEMS = SHARD_ROWS * COLS         # 4,194,304
BLOCK = 32
FD = 8192                               # max free dim per chunk (sbuf tile width)
# chunk schedule: small edge chunks shrink pipeline fill/drain
CHUNK_FDS = [2048, 2048, 4096, 8192, 8192, 4096, 2048, 2048]
assert sum(CHUNK_FDS) * 128 == SHARD_ELEMS
import os as _os
DEVICE_Q = _os.environ.get("BSR_DEVICE_Q", "1") == "1"  # False: host decodes q = VALUES[ord]

_T = [float(np.float32(b) + (32 if b < 2 else 64) * np.spacing(np.float32(b)))
      for b in (0.25, 0.75, 1.25, 1.75, 2.5, 3.5, 5.0)]
T1, T2, T3, T4, T5, T6, T7 = _T
VALUES = np.array([0.0, 0.5, 1.0, 1.5, 2.0, 3.0, 4.0, 6.0], dtype=np.float32)

# ---------------------------------------------------------------- custom ops
def _register_op(name, spec, subdim=False):
    if name in DO._SUB_OPCODE_FOR_NAME:          # idempotent across re-imports
        return next(op for op in DO.OPS if op.name == name)
    row = DO._CUSTOM_DVE_ROW_BASE + len(DO.OPS)
    shas = {}
    for ver in ("v3", "v4"):
        sc = DveOpSpec(name=name, opcode=row, uops=lower(spec, ver=ver),
                       rd1_en=_has_src1(spec))
        shas[ver] = sc.sha(ver)
    op = DO.DveOp(name, spec, subdim=subdim, uops_sha=shas)
    DO.OPS.append(op)
    DO._SUB_OPCODE_FOR_NAME[name] = row
    return op


def _absn(x):
    return Bin(AluOp.ABSOLUTE_VALUE, x, Zero)


P_A = _register_op("BSR_ABS_ADD", Spec(
    body=_absn(Src0) + Src1,
    reference=lambda in0, in1, s0, s1, imm2: (np.abs(in0) + in1).astype(np.float32),
))
P_S = _register_op("BSR_SUM_LO", Spec(
    body=_spill_c3_to_src1((Src0 > C0) + (Src0 > C1) + (Src0 > C2) + (Src0 > C3)),
    reference=lambda in0, in1, s0, s1, imm2: (
        (in0 > s0).astype(np.float32) + (in0 > s1) + (in0 > imm2) + (in0 > in1)
    ).astype(np.float32),
))
P_ORD = _register_op("BSR_ORD", Spec(
    body=Src1 + (Src0 > C0) + (Src0 > C1) + (Src0 > C2),
    reference=lambda in0, in1, s0, s1, imm2: (
        in1 + (in0 > s0) + (in0 > s1) + (in0 > imm2)
    ).astype(np.float32),
))
P_Q = _register_op("BSR_VAL", Spec(
    body=(Src0 + relu(Src0 - C0)) * C1 + (Src0 > C2),
    reference=lambda in0, in1, s0, s1, imm2: (
        (in0 + np.maximum(in0 - s0, 0.0)) * s1 + (in0 > imm2)
    ).astype(np.float32),
))

# ---------------------------------------------------------------- bass module
_NC_CACHE = {}


def _ap(t, offset, ap):
    return bass.AP(tensor=getattr(t, "tensor", t), offset=offset, ap=ap)


def build_nc():
    if "nc" in _NC_CACHE:
        return _NC_CACHE["nc"]
    nc = bacc.Bacc(None, target_bir_lowering=False)
    x = nc.dram_tensor("x", [SHARD_ELEMS], mybir.dt.float32, kind="ExternalInput")
    d = nc.dram_tensor("d", [SHARD_ELEMS // BLOCK], mybir.dt.float32,
                       kind="ExternalInput")
    q = nc.dram_tensor("q", [SHARD_ELEMS], mybir.dt.bfloat16, kind="ExternalOutput")
    o = nc.dram_tensor("o", [SHARD_ELEMS], mybir.dt.uint8, kind="ExternalOutput")

    DBMAX = FD // BLOCK
    xs = [nc.alloc_sbuf_tensor(f"xs{s}", [128, FD], mybir.dt.float32).ap()
          for s in range(2)]
    ds = [nc.alloc_sbuf_tensor(f"ds{s}", [128, DBMAX], mybir.dt.float32).ap()
          for s in range(2)]
    as_ = nc.alloc_sbuf_tensor("as_", [128, FD], mybir.dt.float32).ap()
    qs = [nc.alloc_sbuf_tensor(f"qs{s}", [128, FD], mybir.dt.bfloat16).ap()
          for s in range(2)]
    os_ = [nc.alloc_sbuf_tensor(f"os{s}", [128, FD], mybir.dt.uint8).ap()
           for s in range(2)]
    ss = nc.alloc_sbuf_tensor("ss", [128, FD], mybir.dt.float32).ap()
    c4 = nc.alloc_sbuf_tensor("c4", [128, 1], mybir.dt.float32).ap()

    offs = [0]
    for f in CHUNK_FDS:
        offs.append(offs[-1] + 128 * f)
    NCH = len(CHUNK_FDS)
    n_store_dma = 2 if DEVICE_Q else 1

    # Per-slot load/store sems: DMA completions from different chunks land
    # out of order, so one shared counter would release a consumer while the
    # current chunk's transfer is still in flight. Within one slot, chunks
    # are two apart and the pipeline (asem/wsem gates) guarantees ordering.
    with (
        nc.semaphore("ldsem0") as ldsem0,
        nc.semaphore("ldsem1") as ldsem1,
        nc.semaphore("stsem0") as stsem0,
        nc.semaphore("stsem1") as stsem1,
        nc.semaphore("asem") as asem,     # P_A completions
        nc.semaphore("wsem") as wsem,     # chunk-done (last DVE op) completions
        nc.Block() as block,
    ):
        ldsem = [ldsem0, ldsem1]
        stsem = [stsem0, stsem1]

        @block.sync
        def _(sync):
            for i in range(NCH + 1):
                if i < NCH:
                    s = i % 2
                    fd = CHUNK_FDS[i]
                    db = fd // BLOCK
                    if i >= 2:
                        sync.wait_ge(asem, i - 1)
                    sync.dma_start(
                        out=ds[s][:, :db],
                        in_=_ap(d, offs[i] // BLOCK, [[db, 128], [1, db]]),
                    ).then_inc(ldsem[s], 16)
                    sync.dma_start(
                        out=xs[s][:, :fd],
                        in_=_ap(x, offs[i], [[fd, 128], [1, fd]]),
                    ).then_inc(ldsem[s], 16)
                if i >= 1:
                    j = i - 1
                    s = j % 2
                    fd = CHUNK_FDS[j]
                    sync.wait_ge(wsem, j + 1)
                    if DEVICE_Q:
                        sync.dma_start(
                            out=_ap(q, offs[j], [[fd, 128], [1, fd]]),
                            in_=qs[s][:, :fd],
                        ).then_inc(stsem[s], 16)
                    sync.dma_start(
                        out=_ap(o, offs[j], [[fd, 128], [1, fd]]),
                        in_=os_[s][:, :fd],
                    ).then_inc(stsem[s], 16)
            sync.wait_ge(stsem0, 16 * n_store_dma * ((NCH + 1) // 2))
            sync.wait_ge(stsem1, 16 * n_store_dma * (NCH // 2))

        @block.vector
        def _(vector):
            vector.memset(c4[:], T4)
            for i in range(NCH):
                s = i % 2
                fd = CHUNK_FDS[i]
                db = fd // BLOCK
                vector.wait_ge(ldsem[s], 32 * (i // 2 + 1))
                nc.vector._custom_dve(
                    P_A,
                    out=_ap(as_, 0, [as_.ap[0], [BLOCK, db], [1, BLOCK]]),
                    in0=_ap(xs[s], 0, [xs[s].ap[0], [BLOCK, db], [1, BLOCK]]),
                    in1=_ap(ds[s], 0, [ds[s].ap[0], [1, db], [0, BLOCK]]),
                ).then_inc(asem, 1)
                nc.vector._custom_dve(
                    P_S, out=ss[:, :fd], in0=as_[:, :fd], in1=c4[:],
                    s0=T1, s1=T2, imm2=T3,
                )
                if i >= 2:
                    vector.wait_ge(stsem[s], 16 * n_store_dma * (i // 2))
                last = nc.vector._custom_dve(
                    P_ORD, out=os_[s][:, :fd], in0=as_[:, :fd], in1=ss[:, :fd],
                    s0=T5, s1=T6, imm2=T7,
                )
                if DEVICE_Q:
                    last = nc.vector._custom_dve(
                        P_Q, out=qs[s][:, :fd], in0=os_[s][:, :fd],
                        s0=4.0, s1=0.5, imm2=6.5,
                    )
                last.then_inc(wsem, 1)

    nc.compile()
    _NC_CACHE["nc"] = nc
    return nc


# ---------------------------------------------------------------- host entry
def _delta_device(delta_raw):
    """0.5*tanh on the default jax backend — bit-matches the oracle's eager
    computation (backend tanh differs from libm)."""
    import jax.numpy as jnp
    return np.asarray(0.5 * jnp.tanh(jnp.asarray(np.asarray(delta_raw))))


def _install_trace_shim():
    """Optional: register the axon NTFF profiling hook so _trace=True works
    in containers whose antenv lacks axon_hooks. No-op on failure."""
    import sys, types
    if "antenv.axon_hooks" in sys.modules:
        return
    try:
        from trn_agent_boot.trn_boot import _ntff_profile_via_ctypes
        hook = _ntff_profile_via_ctypes("/opt/axon/libaxon_pjrt.so")
        mod = types.ModuleType("antenv.axon_hooks")
        mod.get_axon_ntff_profile_hook = lambda: hook
        mod.set_axon_ntff_profile_hook = lambda h: None
        sys.modules["antenv.axon_hooks"] = mod
    except Exception:
        pass


def kernel(x_scaled, delta_raw, _trace=False):
    if _trace:
        _install_trace_shim()
    x_scaled = np.ascontiguousarray(np.asarray(x_scaled), dtype=np.float32)
    delta = _delta_device(delta_raw).astype(np.float32, copy=False)

    nc = build_nc()
    in_maps = []
    for c in range(N_CORES):
        xsh = x_scaled[c * SHARD_ROWS:(c + 1) * SHARD_ROWS].reshape(-1)
        dsh = delta[c * (SHARD_ELEMS // BLOCK):(c + 1) * (SHARD_ELEMS // BLOCK)]
        in_maps.append({"x": xsh, "d": np.ascontiguousarray(dsh)})

    res = run_bass_kernel_spmd(nc, in_maps, list(range(N_CORES)), trace=_trace)

    o = np.concatenate([res.results[c]["o"].astype(np.int32)
                        for c in range(N_CORES)])
    o = o.reshape(ROWS, COLS)
    if DEVICE_Q:
        q = np.concatenate([res.results[c]["q"].astype(np.float32)
                            for c in range(N_CORES)]).reshape(ROWS, COLS)
    else:
        q = VALUES[o]
    out = (q, o)
    if _trace:
        return out, res
    return out



# revision 3
# speedup vs baseline: 1.8805x; 1.8805x over previous
"""nn_BlockSharedRounding Trainium2 kernel.

Computes the forward of the block-shared soft rounding reference:
    a   = |x| + 0.5*tanh(delta_raw) per 32-block
    ord = searchsorted(BOUNDS, a, 'left')
    q   = VALUES[ord]                       (== abs_mix forward value)

Strategy: data-parallel over 8 NeuronCores (rows of x). Per core, a raw
Bass kernel streams [128, fd] fp32 chunks through ONE fused custom DVE
op per chunk:

    B = ((|x + x| + d') + (|x + x| + d'))   with d' = tanh(delta_raw) + 1.25
      = 4*(|x| + 0.5*tanh(delta_raw)) + 2.5

written to a uint8 output. The DVE's fp32->uint8 output conversion is
round-to-nearest-even with saturation (verified on HW), so the byte b
pins 4*a to the half-open interval [b-3, b-2). All scaled bin edges
{1,3,5,7,10,14,20} are integers, i.e. interval endpoints, so every byte
maps to a unique bin: the uint8 quantizer performs all 7 searchsorted
comparisons for free. The host decodes ord and q from the byte stream
with 256-entry LUTs (pure re-encoding; every element's bin is fully
determined on device). Misclassification is possible only for elements
within ~1 ulp of a bin edge (measure-zero for random normal inputs,
and far inside the 2e-2 rel-err budget).

HBM traffic per core: 16.78 MB x in + 0.52 MB d in + 4.19 MB out
= 21.5 MB, vs 29.9 MB for the previous 4-pass bf16+u8 kernel; DVE work
drops from 4 full passes to 1.
"""
import numpy as np

import concourse.bass as bass
import concourse.bacc as bacc
import concourse.mybir as mybir
import concourse.dve_ops as DO
from concourse.dve_uop import DveOpSpec
from concourse.dve_spec import (
    Spec, Src0, Src1, Zero, Bin, AluOp, lower, _has_src1,
)
from concourse.bass_utils import run_bass_kernel_spmd

# ---------------------------------------------------------------- constants
N_CORES = 8
ROWS, COLS = 4096, 8192
SHARD_ROWS = ROWS // N_CORES            # 512
SHARD_ELEMS = SHARD_ROWS * COLS         # 4,194,304
BLOCK = 32
FD = 8192                               # max free dim per chunk
# chunk schedule: small edge chunks shrink pipeline fill/drain
CHUNK_FDS = [2048, 2048, 4096, 8192, 8192, 4096, 2048, 2048]
assert sum(CHUNK_FDS) * 128 == SHARD_ELEMS

D_OFFSET = 1.25                         # d' = tanh(raw) + D_OFFSET
VALUES = np.array([0.0, 0.5, 1.0, 1.5, 2.0, 3.0, 4.0, 6.0], dtype=np.float32)
_EDGES4 = np.array([1, 3, 5, 7, 10, 14, 20])   # 4*BOUNDS, exact integers
# byte b <=> 4a in [b-3, b-2)  =>  ord = #edges <= b-3
ORD_LUT = np.array([int(np.searchsorted(_EDGES4, b - 3, side="right"))
                    for b in range(256)], dtype=np.uint8)
Q_LUT = VALUES[ORD_LUT]                 # float32 [256]

# ---------------------------------------------------------------- custom op
def _register_op(name, spec, subdim=False):
    if name in DO._SUB_OPCODE_FOR_NAME:          # idempotent across re-imports
        return next(op for op in DO.OPS if op.name == name)
    row = DO._CUSTOM_DVE_ROW_BASE + len(DO.OPS)
    shas = {}
    for ver in ("v3", "v4"):
        sc = DveOpSpec(name=name, opcode=row, uops=lower(spec, ver=ver),
                       rd1_en=_has_src1(spec))
        shas[ver] = sc.sha(ver)
    op = DO.DveOp(name, spec, subdim=subdim, uops_sha=shas)
    DO.OPS.append(op)
    DO._SUB_OPCODE_FOR_NAME[name] = row
    return op


def _body():
    t = Src0 + Src0                      # 2x
    u = Bin(AluOp.ABSOLUTE_VALUE, t, Zero)   # |2x|
    a = u + Src1                         # |2x| + d'
    return a + a                         # 4|x| + 2*tanh + 2.5


P_BIN = _register_op("BSR_BIN4", Spec(
    body=_body(),
    reference=lambda in0, in1, s0, s1, imm2:
        ((np.abs(in0 + in0) + in1) * 2).astype(np.float32),
))

# ---------------------------------------------------------------- bass module
_NC_CACHE = {}


def _ap(t, offset, ap):
    return bass.AP(tensor=getattr(t, "tensor", t), offset=offset, ap=ap)


def build_nc():
    if "nc" in _NC_CACHE:
        return _NC_CACHE["nc"]
    nc = bacc.Bacc(None, target_bir_lowering=False)
    x = nc.dram_tensor("x", [SHARD_ELEMS], mybir.dt.float32, kind="ExternalInput")
    d = nc.dram_tensor("d", [SHARD_ELEMS // BLOCK], mybir.dt.float32,
                       kind="ExternalInput")
    b = nc.dram_tensor("b", [SHARD_ELEMS], mybir.dt.uint8, kind="ExternalOutput")

    DBMAX = FD // BLOCK
    xs = [nc.alloc_sbuf_tensor(f"xs{s}", [128, FD], mybir.dt.float32).ap()
          for s in range(2)]
    ds = [nc.alloc_sbuf_tensor(f"ds{s}", [128, DBMAX], mybir.dt.float32).ap()
          for s in range(2)]
    os_ = [nc.alloc_sbuf_tensor(f"os{s}", [128, FD], mybir.dt.uint8).ap()
           for s in range(2)]

    offs = [0]
    for f in CHUNK_FDS:
        offs.append(offs[-1] + 128 * f)
    NCH = len(CHUNK_FDS)

    # Per-slot load/store sems: DMA completions from different chunks land
    # out of order, so one shared counter would release a consumer while the
    # current chunk's transfer is still in flight. Within one slot, chunks
    # are two apart and the wsem gates guarantee ordering.
    with (
        nc.semaphore("ldsem0") as ldsem0,
        nc.semaphore("ldsem1") as ldsem1,
        nc.semaphore("stsem0") as stsem0,
        nc.semaphore("stsem1") as stsem1,
        nc.semaphore("wsem") as wsem,     # chunk-done (DVE op) completions
        nc.Block() as block,
    ):
        ldsem = [ldsem0, ldsem1]
        stsem = [stsem0, stsem1]

        @block.sync
        def _(sync):
            for i in range(NCH + 1):
                if i < NCH:
                    s = i % 2
                    fd = CHUNK_FDS[i]
                    db = fd // BLOCK
                    if i >= 2:
                        # xs/ds slot reuse: DVE of chunk i-2 must be done
                        sync.wait_ge(wsem, i - 1)
                    sync.dma_start(
                        out=ds[s][:, :db],
                        in_=_ap(d, offs[i] // BLOCK, [[db, 128], [1, db]]),
                    ).then_inc(ldsem[s], 16)
                    sync.dma_start(
                        out=xs[s][:, :fd],
                        in_=_ap(x, offs[i], [[fd, 128], [1, fd]]),
                    ).then_inc(ldsem[s], 16)
                if i >= 1:
                    j = i - 1
                    s = j % 2
                    fd = CHUNK_FDS[j]
                    sync.wait_ge(wsem, j + 1)
                    sync.dma_start(
                        out=_ap(b, offs[j], [[fd, 128], [1, fd]]),
                        in_=os_[s][:, :fd],
                    ).then_inc(stsem[s], 16)
            sync.wait_ge(stsem0, 16 * ((NCH + 1) // 2))
            sync.wait_ge(stsem1, 16 * (NCH // 2))

        @block.vector
        def _(vector):
            for i in range(NCH):
                s = i % 2
                fd = CHUNK_FDS[i]
                db = fd // BLOCK
                vector.wait_ge(ldsem[s], 32 * (i // 2 + 1))
                if i >= 2:
                    # os_ slot reuse: store of chunk i-2 must be done
                    vector.wait_ge(stsem[s], 16 * (i // 2))
                nc.vector._custom_dve(
                    P_BIN,
                    out=_ap(os_[s], 0, [os_[s].ap[0], [BLOCK, db], [1, BLOCK]]),
                    in0=_ap(xs[s], 0, [xs[s].ap[0], [BLOCK, db], [1, BLOCK]]),
                    in1=_ap(ds[s], 0, [ds[s].ap[0], [1, db], [0, BLOCK]]),
                ).then_inc(wsem, 1)

    nc.compile()
    _NC_CACHE["nc"] = nc
    return nc


# ---------------------------------------------------------------- host entry
def _install_trace_shim():
    """Optional: register the axon NTFF profiling hook so _trace=True works
    in containers whose antenv lacks axon_hooks. No-op on failure."""
    import sys, types
    if "antenv.axon_hooks" in sys.modules:
        return
    try:
        from trn_agent_boot.trn_boot import _ntff_profile_via_ctypes
        hook = _ntff_profile_via_ctypes("/opt/axon/libaxon_pjrt.so")
        mod = types.ModuleType("antenv.axon_hooks")
        mod.get_axon_ntff_profile_hook = lambda: hook
        mod.set_axon_ntff_profile_hook = lambda h: None
        sys.modules["antenv.axon_hooks"] = mod
    except Exception:
        pass


def kernel(x_scaled, delta_raw, _trace=False):
    if _trace:
        _install_trace_shim()
    x_scaled = np.ascontiguousarray(np.asarray(x_scaled), dtype=np.float32)
    delta_raw = np.asarray(delta_raw)
    dprep = (np.tanh(delta_raw.astype(np.float32)) + np.float32(D_OFFSET)
             ).astype(np.float32)

    nc = build_nc()
    in_maps = []
    nblk = SHARD_ELEMS // BLOCK
    for c in range(N_CORES):
        xsh = x_scaled[c * SHARD_ROWS:(c + 1) * SHARD_ROWS].reshape(-1)
        dsh = dprep[c * nblk:(c + 1) * nblk]
        in_maps.append({"x": xsh, "d": np.ascontiguousarray(dsh)})

    res = run_bass_kernel_spmd(nc, in_maps, list(range(N_CORES)), trace=_trace)

    bb = np.concatenate([res.results[c]["b"] for c in range(N_CORES)])
    o = ORD_LUT[bb].astype(np.int32).reshape(ROWS, COLS)
    q = Q_LUT[bb].reshape(ROWS, COLS)
    out = (q, o)
    if _trace:
        return out, res
    return out


# revision 8
# speedup vs baseline: 2.4017x; 1.2771x over previous
"""nn_BlockSharedRounding Trainium2 kernel.

Computes the forward of the block-shared soft rounding reference:
    a   = |x| + 0.5*tanh(delta_raw) per 32-block
    ord = searchsorted(BOUNDS, a, 'left')
    q   = VALUES[ord]                       (== abs_mix forward value)

Strategy: data-parallel over 8 NeuronCores (rows of x). Per core, a raw
Bass kernel streams [128, fd] fp32 chunks through ONE fused custom DVE
op per chunk:

    B = ((|x + x| + d') + (|x + x| + d'))   with d' = tanh(delta_raw) + 1.25
      = 4*(|x| + 0.5*tanh(delta_raw)) + 2.5

written to a uint8 output. The DVE's fp32->uint8 output conversion is
round-to-nearest-even with saturation (verified on HW), so the byte b
pins 4*a to the half-open interval [b-3, b-2). All scaled bin edges
{1,3,5,7,10,14,20} are integers, i.e. interval endpoints, so every byte
maps to a unique bin: the uint8 quantizer performs all 7 searchsorted
comparisons for free. The host decodes ord and q from the byte stream
with 256-entry LUTs (pure re-encoding; every element's bin is fully
determined on device). Misclassification is possible only for elements
within ~1 ulp of a bin edge (measure-zero for random normal inputs,
and far inside the 2e-2 rel-err budget).

HBM traffic per core: 16.78 MB x in + 0.52 MB d in + 4.19 MB out
= 21.5 MB, vs 29.9 MB for the previous 4-pass bf16+u8 kernel; DVE work
drops from 4 full passes to 1.
"""
import numpy as np

import concourse.bass as bass
import concourse.bacc as bacc
import concourse.mybir as mybir
import concourse.dve_ops as DO
from concourse.dve_uop import DveOpSpec
from concourse.dve_spec import (
    Spec, Src0, Src1, Zero, Bin, AluOp, lower, _has_src1,
)
from concourse.bass_utils import run_bass_kernel_spmd

# ---------------------------------------------------------------- constants
N_CORES = 8
ROWS, COLS = 4096, 8192
SHARD_ROWS = ROWS // N_CORES            # 512
SHARD_ELEMS = SHARD_ROWS * COLS         # 4,194,304
BLOCK = 32
# Chunk schedule: every chunk gets its own SBUF buffers (no reuse, no
# inter-stage gating), so all loads are issued up-front and the DMA queue
# never starves. Small tail chunks shrink the end-of-pipeline drain.
CHUNK_FDS = [4096, 4096, 4096, 4096, 4096, 4096, 4096, 2048, 1024, 1024]
assert sum(CHUNK_FDS) * 128 == SHARD_ELEMS

D_OFFSET = 1.25                         # d' = tanh(raw) + D_OFFSET
VALUES = np.array([0.0, 0.5, 1.0, 1.5, 2.0, 3.0, 4.0, 6.0], dtype=np.float32)
_EDGES4 = np.array([1, 3, 5, 7, 10, 14, 20])   # 4*BOUNDS, exact integers
# byte b <=> 4a in [b-3, b-2)  =>  ord = #edges <= b-3
ORD_LUT = np.array([int(np.searchsorted(_EDGES4, b - 3, side="right"))
                    for b in range(256)], dtype=np.uint8)
Q_LUT = VALUES[ORD_LUT]                 # float32 [256]

# ---------------------------------------------------------------- custom op
def _register_op(name, spec, subdim=False):
    if name in DO._SUB_OPCODE_FOR_NAME:          # idempotent across re-imports
        return next(op for op in DO.OPS if op.name == name)
    row = DO._CUSTOM_DVE_ROW_BASE + len(DO.OPS)
    shas = {}
    for ver in ("v3", "v4"):
        sc = DveOpSpec(name=name, opcode=row, uops=lower(spec, ver=ver),
                       rd1_en=_has_src1(spec))
        shas[ver] = sc.sha(ver)
    op = DO.DveOp(name, spec, subdim=subdim, uops_sha=shas)
    DO.OPS.append(op)
    DO._SUB_OPCODE_FOR_NAME[name] = row
    return op


def _body():
    t = Src0 + Src0                      # 2x
    u = Bin(AluOp.ABSOLUTE_VALUE, t, Zero)   # |2x|
    a = u + Src1                         # |2x| + d'
    return a + a                         # 4|x| + 2*tanh + 2.5


P_BIN = _register_op("BSR_BIN4", Spec(
    body=_body(),
    reference=lambda in0, in1, s0, s1, imm2:
        ((np.abs(in0 + in0) + in1) * 2).astype(np.float32),
))

# ---------------------------------------------------------------- bass module
_NC_CACHE = {}


def _ap(t, offset, ap):
    return bass.AP(tensor=getattr(t, "tensor", t), offset=offset, ap=ap)


def build_nc():
    if "nc" in _NC_CACHE:
        return _NC_CACHE["nc"]
    nc = bacc.Bacc(None, target_bir_lowering=False)
    x = nc.dram_tensor("x", [SHARD_ELEMS], mybir.dt.float32, kind="ExternalInput")
    d = nc.dram_tensor("d", [SHARD_ELEMS // BLOCK], mybir.dt.float32,
                       kind="ExternalInput")
    b = nc.dram_tensor("b", [SHARD_ELEMS], mybir.dt.uint8, kind="ExternalOutput")

    NCH = len(CHUNK_FDS)
    xs = [nc.alloc_sbuf_tensor(f"xs{i}", [128, CHUNK_FDS[i]],
                               mybir.dt.float32).ap() for i in range(NCH)]
    ds = [nc.alloc_sbuf_tensor(f"ds{i}", [128, CHUNK_FDS[i] // BLOCK],
                               mybir.dt.float32).ap() for i in range(NCH)]
    os_ = [nc.alloc_sbuf_tensor(f"os{i}", [128, CHUNK_FDS[i]],
                                mybir.dt.uint8).ap() for i in range(NCH)]

    offs = [0]
    for f in CHUNK_FDS:
        offs.append(offs[-1] + 128 * f)

    # One load-sem per chunk (DMA completions land out of order across
    # chunks, so counters cannot be shared); a single store-completion sem.
    from contextlib import ExitStack
    with ExitStack() as stack:
        ldsem = [stack.enter_context(nc.semaphore(f"ld{i}"))
                 for i in range(NCH)]
        stsem = stack.enter_context(nc.semaphore("stsem"))
        wsem = stack.enter_context(nc.semaphore("wsem"))
        block = stack.enter_context(nc.Block())
        @block.sync
        def _(sync):
            # every chunk has dedicated buffers: issue all loads up-front
            for i in range(NCH):
                fd = CHUNK_FDS[i]
                db = fd // BLOCK
                sync.dma_start(
                    out=ds[i][:],
                    in_=_ap(d, offs[i] // BLOCK, [[db, 128], [1, db]]),
                ).then_inc(ldsem[i], 16)
                sync.dma_start(
                    out=xs[i][:],
                    in_=_ap(x, offs[i], [[fd, 128], [1, fd]]),
                ).then_inc(ldsem[i], 16)
            sync.wait_ge(stsem, 16 * NCH)

        @block.vector
        def _(vector):
            for i in range(NCH):
                fd = CHUNK_FDS[i]
                db = fd // BLOCK
                vector.wait_ge(ldsem[i], 32)
                nc.vector._custom_dve(
                    P_BIN,
                    out=_ap(os_[i], 0, [os_[i].ap[0], [BLOCK, db], [1, BLOCK]]),
                    in0=_ap(xs[i], 0, [xs[i].ap[0], [BLOCK, db], [1, BLOCK]]),
                    in1=_ap(ds[i], 0, [ds[i].ap[0], [1, db], [0, BLOCK]]),
                ).then_inc(wsem, 1)

        @block.scalar
        def _(scalar):
            # stores ride the scalar engine's own DMA queue, separate from
            # the sync engine's load queue
            for i in range(NCH):
                fd = CHUNK_FDS[i]
                scalar.wait_ge(wsem, i + 1)
                scalar.dma_start(
                    out=_ap(b, offs[i], [[fd, 128], [1, fd]]),
                    in_=os_[i][:],
                ).then_inc(stsem, 16)

    nc.compile()
    _NC_CACHE["nc"] = nc
    return nc


# ---------------------------------------------------------------- host entry
def _install_trace_shim():
    """Optional: register the axon NTFF profiling hook so _trace=True works
    in containers whose antenv lacks axon_hooks. No-op on failure."""
    import sys, types
    if "antenv.axon_hooks" in sys.modules:
        return
    try:
        from trn_agent_boot.trn_boot import _ntff_profile_via_ctypes
        hook = _ntff_profile_via_ctypes("/opt/axon/libaxon_pjrt.so")
        mod = types.ModuleType("antenv.axon_hooks")
        mod.get_axon_ntff_profile_hook = lambda: hook
        mod.set_axon_ntff_profile_hook = lambda h: None
        sys.modules["antenv.axon_hooks"] = mod
    except Exception:
        pass


def kernel(x_scaled, delta_raw, _trace=False):
    if _trace:
        _install_trace_shim()
    x_scaled = np.ascontiguousarray(np.asarray(x_scaled), dtype=np.float32)
    delta_raw = np.asarray(delta_raw)
    dprep = (np.tanh(delta_raw.astype(np.float32)) + np.float32(D_OFFSET)
             ).astype(np.float32)

    nc = build_nc()
    in_maps = []
    nblk = SHARD_ELEMS // BLOCK
    for c in range(N_CORES):
        xsh = x_scaled[c * SHARD_ROWS:(c + 1) * SHARD_ROWS].reshape(-1)
        dsh = dprep[c * nblk:(c + 1) * nblk]
        in_maps.append({"x": xsh, "d": np.ascontiguousarray(dsh)})

    res = run_bass_kernel_spmd(nc, in_maps, list(range(N_CORES)), trace=_trace)

    bb = np.concatenate([res.results[c]["b"] for c in range(N_CORES)])
    o = ORD_LUT[bb].astype(np.int32).reshape(ROWS, COLS)
    q = Q_LUT[bb].reshape(ROWS, COLS)
    out = (q, o)
    if _trace:
        return out, res
    return out
